# revision 3
# baseline (speedup 1.0000x reference)
"""Trainium2 Bass kernel v2 for nn_MultiHeadAttn (dense transformer block).

Data-parallel over batch N=8 -> one batch item per NeuronCore. All
activations transposed (channels on partitions, seq on free dim); host does
boundary transposes + channel permutation.

v2 over baseline:
- bf16 matmul operands everywhere (1024-wide moving ops, FWL weight loads)
- quadrant/row-packed matmuls via tile_position inference: q/k/v/proj pairs
  and scores A/B run concurrently on disjoint PE sub-arrays
- exp scale folded via wq; exp outputs bf16 directly from PSUM
- softmax rowsum via ones-column in v (row 64 of u); u drained by DMA with
  partition restacking; one reciprocal per tile-pair
- LN rstd via exp(-0.5*ln(v)) to stay in the natural_log_exp act table set
- LN stats via col-packed ones-matmuls into one PSUM tile post-pass
- FF in bf16 with folded gamma/beta
"""

import numpy as np
import ml_dtypes

import concourse.bacc as bacc
import concourse.mybir as mybir
import concourse.tile as tile
from concourse.bass_utils import run_bass_kernel_spmd

F32 = mybir.dt.float32
F32R = mybir.dt.float32r
BF16 = mybir.dt.bfloat16
AF = mybir.ActivationFunctionType
OP = mybir.AluOpType

S = 1024
D = 768
H = 12
DH = 64
HID = 64
NT = 6  # channel tiles of 128 (2 heads each)
KC = 8  # key chunks of 128
LN_EPS = 1e-5

_CACHE = {}


def build_nc(loop_n=None, debug=False, unroll=1):
    nc = bacc.Bacc("TRN2", target_bir_lowering=False, debug=False)

    xT_d = nc.dram_tensor("xT", [D, S], BF16, kind="ExternalInput")
    wq2_d = nc.dram_tensor("wq2", [128, HID], BF16, kind="ExternalInput")
    wk2_d = nc.dram_tensor("wk2", [128, HID], BF16, kind="ExternalInput")
    wv2_d = nc.dram_tensor("wv2", [128, 65], BF16, kind="ExternalInput")
    wp2_d = nc.dram_tensor("wp2", [DH, DH], BF16, kind="ExternalInput")
    bq2_d = nc.dram_tensor("bq2", [128, 1], F32, kind="ExternalInput")
    bp2_d = nc.dram_tensor("bp2", [128, 1], F32, kind="ExternalInput")
    wffp_d = nc.dram_tensor("wffp", [D, D], BF16, kind="ExternalInput")
    bffp_d = nc.dram_tensor("bffp", [128, NT], F32, kind="ExternalInput")
    onesb_d = nc.dram_tensor("onesb", [128, 1], BF16, kind="ExternalInput")
    negwcol_d = nc.dram_tensor("negwcol", [1, D], BF16, kind="ExternalInput")
    out_d = nc.dram_tensor("out", [D, S], F32, kind="ExternalOutput")
    dbg = {}
    if debug:
        for nm, shp in [
            ("dbg_qT", [128, S]), ("dbg_kT", [128, S]), ("dbg_v", [128, 130]),
            ("dbg_eA", [128, S]), ("dbg_u2", [128, S]), ("dbg_rs2", [2, S]),
            ("dbg_a", [128, S]), ("dbg_mean", [1, S]), ("dbg_rstd", [1, S]),
            ("dbg_nr", [128, S]),
        ]:
            dbg[nm] = nc.dram_tensor(nm, shp, F32, kind="ExternalOutput")

    with tile.TileContext(nc) as tc:
        # weights loaded once, outside the timing loop
        cpool_cm = tc.tile_pool(name="const", bufs=1)
        cpool = cpool_cm.__enter__()

        def load(dram, shape, dt):
            r = cpool.tile(shape, dt, name=f"r_{dram.name}")
            nc.sync.dma_start(r[:], dram[:])
            return r

        wq2r = load(wq2_d, [128, HID], BF16)
        wk2r = load(wk2_d, [128, HID], BF16)
        wv2r = load(wv2_d, [128, 65], BF16)
        wp2r = load(wp2_d, [DH, DH], BF16)
        onesb = load(onesb_d, [128, 1], BF16)
        negwcol = load(negwcol_d, [1, D], BF16)
        bq2 = load(bq2_d, [128, 1], F32)
        bp2 = load(bp2_d, [128, 1], F32)
        wffr = []
        for t in range(NT):
            r = cpool.tile([128, D], BF16, name=f"wffr{t}")
            nc.sync.dma_start(r[:], wffp_d[128 * t : 128 * (t + 1), :])
            wffr.append(r)
        bff_all = cpool.tile([128, NT], F32, name="bff_all")
        nc.sync.dma_start(bff_all[:], bffp_d[:])
        gb = [bff_all[:, t : t + 1] for t in range(NT)]

        pools_cm = [
            tc.tile_pool(name="atile", bufs=1),
            tc.tile_pool(name="psS", bufs=2, space="PSUM"),
            tc.tile_pool(name="psU", bufs=2, space="PSUM"),
            tc.tile_pool(name="qkv", bufs=1),
            tc.tile_pool(name="xr", bufs=1),
            tc.tile_pool(name="p2w", bufs=2),
            tc.tile_pool(name="p3w", bufs=2),
            tc.tile_pool(name="p3s", bufs=1),
        ]
        apool, psS, psU, qkvpool, xrpool, w2, w3, s3 = (
            cm.__enter__() for cm in pools_cm
        )

        def body(_i=None):
            if True:
                aT = []
                if True:
                    qTr = [None] * NT
                    kTr = [None] * NT
                    vr = [None] * NT

                    def proj_pair(t):
                        xr = xrpool.tile([128, S], BF16, name="xr", tag="xr", bufs=2)
                        nc.sync.dma_start(xr[:], xT_d[128 * t : 128 * (t + 1), :])
                        # q: two quadrant-packed matmuls (0,0) and (64,64)
                        psq = psU.tile([128, S], F32, name="psq", tag="u")
                        for qh in range(2):
                            sl = slice(512 * qh, 512 * (qh + 1))
                            nc.tensor.matmul(
                                psq[0:64, sl], wq2r[0:64, :], xr[0:64, sl],
                                start=True, stop=True,
                            )
                            nc.tensor.matmul(
                                psq[64:128, sl], wq2r[64:128, :], xr[64:128, sl],
                                start=True, stop=True,
                            )
                        q = qkvpool.tile([128, S], BF16, name=f"qTr{t}")
                        nc.vector.tensor_scalar_add(q[:], psq[:], bq2[:])
                        qTr[t] = q
                        # k: same packing
                        psk = psU.tile([128, S], F32, name="psk", tag="u")
                        for qh in range(2):
                            sl = slice(512 * qh, 512 * (qh + 1))
                            nc.tensor.matmul(
                                psk[0:64, sl], wk2r[0:64, :], xr[0:64, sl],
                                start=True, stop=True,
                            )
                            nc.tensor.matmul(
                                psk[64:128, sl], wk2r[64:128, :], xr[64:128, sl],
                                start=True, stop=True,
                            )
                        # no bk: softmax is invariant to the constant q.bk
                        # shift per row; bv is folded through proj into bp2
                        kk = qkvpool.tile([128, S], BF16, name=f"kTr{t}")
                        nc.vector.tensor_copy(kk[:], psk[:])
                        kTr[t] = kk
                        # v natural layout: per-head tiles; each psum take
                        # holds 4 seq-chunks of one head (single row group)
                        vsA = qkvpool.tile([128, 65 * KC], BF16, name=f"vrA{t}")
                        vsB = qkvpool.tile([128, 65 * KC], BF16, name=f"vrB{t}")
                        for half in range(2):
                            vpsA = psU.tile([128, 260], F32, name="vpsA", tag="u")
                            vpsB = psU.tile([128, 260], F32, name="vpsB", tag="u")
                            for jj in range(4):
                                j = 4 * half + jj
                                nc.tensor.matmul(
                                    vpsA[:, 65 * jj : 65 * (jj + 1)],
                                    xr[0:64, 128 * j : 128 * (j + 1)],
                                    wv2r[0:64, :],
                                    start=True, stop=True,
                                )
                                nc.tensor.matmul(
                                    vpsB[:, 65 * jj : 65 * (jj + 1)],
                                    xr[64:128, 128 * j : 128 * (j + 1)],
                                    wv2r[64:128, :],
                                    start=True, stop=True,
                                )
                            nc.vector.tensor_copy(
                                vsA[:, 260 * half : 260 * (half + 1)], vpsA[:]
                            )
                            nc.vector.tensor_copy(
                                vsB[:, 260 * half : 260 * (half + 1)], vpsB[:]
                            )
                        nc.vector.memset(vsA[:, 64 : 65 * KC : 65], 1.0)
                        nc.vector.memset(vsB[:, 64 : 65 * KC : 65], 1.0)
                        vr[t] = (vsA, vsB)
                        if debug and t == 0:
                            nc.gpsimd.dma_start(dbg["dbg_qT"][:], qTr[0][:])
                            nc.gpsimd.dma_start(dbg["dbg_kT"][:], kTr[0][:])
                            nc.gpsimd.dma_start(dbg["dbg_v"][:, 0:65], vr[0][0][:, 0:65])
                            nc.gpsimd.dma_start(dbg["dbg_v"][:, 65:130], vr[0][1][:, 0:65])

                    for t in range(NT):
                        proj_pair(t)
                    pend = [None]  # deferred proj/normalize of tile t-1
                    sqT = []
                    for t in range(NT):
                        uA = psU.tile([65, S], F32, name="uA", tag="u")
                        uB = psU.tile([65, S], F32, name="uB", tag="u")
                        eAp = [None]
                        eBp = [None]

                        def scores_exp(kc, which, eout):
                            sx = psS.tile([128, S], F32, name=f"s{which}", tag="s")
                            lo = 0 if which == "A" else 64
                            for qh in range(2):
                                sl = slice(512 * qh, 512 * (qh + 1))
                                nc.tensor.matmul(
                                    sx[:, sl],
                                    kTr[t][lo : lo + 64, 128 * kc : 128 * (kc + 1)],
                                    qTr[t][lo : lo + 64, sl],
                                    start=True, stop=True,
                                )
                            e = w2.tile([128, S], BF16, name=f"e{which}",
                                        tag=f"e{which}", bufs=2)
                            nc.scalar.activation(e[:], sx[:], AF.Exp)
                            eout[0] = e

                        def av(kc, which, e, u):
                            st = kc == 0
                            fin = kc == KC - 1
                            vt = vr[t][0 if which == "A" else 1]
                            for qh in range(2):
                                sl = slice(512 * qh, 512 * (qh + 1))
                                nc.tensor.matmul(
                                    u[:, sl],
                                    vt[:, 65 * kc : 65 * (kc + 1)],
                                    e[:, sl],
                                    start=st, stop=fin,
                                )

                        for kc in range(KC):
                            eA_prev, eB_prev = eAp[0], eBp[0]
                            scores_exp(kc, "A", eAp)
                            if kc > 0:
                                av(kc - 1, "A", eA_prev, uA)
                            scores_exp(kc, "B", eBp)
                            if kc > 0:
                                av(kc - 1, "B", eB_prev, uB)
                            if debug and t == 0 and kc == 0:
                                nc.gpsimd.dma_start(dbg["dbg_eA"][:], eAp[0][:])
                            if kc == 2 and pend[0] is not None:
                                pend[0]()
                                pend[0] = None
                        av(KC - 1, "A", eAp[0], uA)
                        av(KC - 1, "B", eBp[0], uB)

                        # drain u hidden rows to bf16; broadcast raw rowsums
                        # straight from PSUM row 64 (no reciprocal: divide)
                        uAs = w2.tile([65, S], BF16, name="uAs", tag="uAs", bufs=2)
                        nc.vector.tensor_copy(uAs[:], uA[:])
                        uBs = w2.tile([65, S], BF16, name="uBs", tag="uBs", bufs=2)
                        nc.vector.tensor_copy(uBs[:], uB[:])
                        # partition_broadcast reads partition 0 only (HW):
                        # DMA-shift rowsum rows to partition 0 first
                        rsA = w2.tile([1, S], BF16, name="rsA", tag="rsA", bufs=2)
                        nc.gpsimd.dma_start(rsA[:], uAs[64:65, :])
                        rsB = w2.tile([1, S], BF16, name="rsB", tag="rsB", bufs=2)
                        nc.gpsimd.dma_start(rsB[:], uBs[64:65, :])
                        rbA = w2.tile([128, S], BF16, name="rbA", tag="rbA", bufs=2)
                        nc.gpsimd.partition_broadcast(rbA[:], rsA[:])
                        rbB = w2.tile([128, S], BF16, name="rbB", tag="rbB", bufs=2)
                        nc.gpsimd.partition_broadcast(rbB[:], rsB[:])
                        rbw = w2.tile([128, S], BF16, name="rbw", tag="rbw", bufs=2)
                        nc.gpsimd.dma_start(rbw[0:64, :], rbA[0:64, :])
                        nc.gpsimd.dma_start(rbw[64:128, :], rbB[64:128, :])
                        rb = w2.tile([128, S], F32, name="rb", tag="rb", bufs=2)
                        nc.vector.reciprocal(rb[:], rbw[:])
                        if debug and t == 0:
                            nc.gpsimd.dma_start(
                                dbg["dbg_u2"][0:65, :], uAs[:]
                            )
                            nc.gpsimd.dma_start(dbg["dbg_rs2"][0:1, :], rb[0:1, :])
                            nc.gpsimd.dma_start(dbg["dbg_rs2"][1:2, :], rb[64:65, :])

                        def fin(t=t, uAs=uAs, uBs=uBs, rb=rb):
                            # proj: col-packed pair (0,0) / (0,64), bf16
                            p2 = psS.tile([128, S], F32, name="p2", tag="s")
                            for qh in range(2):
                                sl = slice(512 * qh, 512 * (qh + 1))
                                nc.tensor.matmul(
                                    p2[0:64, sl], wp2r[:], uAs[0:64, sl],
                                    start=True, stop=True,
                                )
                                nc.tensor.matmul(
                                    p2[64:128, sl], wp2r[:], uBs[0:64, sl],
                                    start=True, stop=True,
                                )
                            a1 = w2.tile([128, S], BF16, name="a1", tag="a1",
                                         bufs=2)
                            nc.vector.tensor_mul(a1[:], p2[:], rb[:])
                            at = apool.tile([128, S], BF16, name=f"aT{t}")
                            nc.vector.tensor_scalar_add(at[:], a1[:], bp2[:])
                            sq = apool.tile([128, S], BF16, name=f"sq{t}")
                            nc.vector.tensor_mul(sq[:], at[:], at[:])
                            if debug and t == 0:
                                nc.gpsimd.dma_start(
                                    dbg["dbg_a"][:], at[:]
                                )
                            aT.append(at)
                            sqT.append(sq)

                        pend[0] = fin
                    pend[0]()
                    pend[0] = None

                # ---- LN stats (col-packed ones matmuls) ----
                if True:
                    stats = psS.tile([33, S], F32, name="stats", tag="s")
                    for t in range(NT):
                        st = t == 0
                        fin = t == NT - 1
                        for qh in range(2):
                            sl = slice(512 * qh, 512 * (qh + 1))
                            nc.tensor.matmul(
                                stats[0:1, sl], onesb[:], aT[t][:, sl],
                                start=st, stop=fin, tile_position=(0, 0),
                            )
                            nc.tensor.matmul(
                                stats[32:33, sl], onesb[:], sqT[t][:, sl],
                                start=st, stop=fin, tile_position=(0, 32),
                            )
                    mean = s3.tile([1, S], BF16, name="mean")
                    nc.vector.tensor_scalar_mul(mean[:], stats[0:1, :], 1.0 / D)
                    msq = s3.tile([1, S], F32, name="msq")
                    nc.vector.tensor_scalar_mul(msq[:], stats[32:33, :], 1.0 / D)
                    m2 = s3.tile([1, S], F32, name="m2")
                    nc.scalar.activation(m2[:], mean[:], AF.Square)
                    vpe = s3.tile([1, S], F32, name="vpe")
                    nc.vector.scalar_tensor_tensor(
                        vpe[:], msq[:], LN_EPS, m2[:], op0=OP.add, op1=OP.subtract
                    )
                    lv = s3.tile([1, S], F32, name="lv")
                    nc.scalar.activation(lv[:], vpe[:], AF.Ln)
                    rstd = s3.tile([1, S], BF16, name="rstd")
                    nc.scalar.activation(rstd[:], lv[:], AF.Exp, scale=-0.5)
                    if debug:
                        nc.gpsimd.dma_start(
                            dbg["dbg_mean"][:], mean[:]
                        )
                        nc.gpsimd.dma_start(
                            dbg["dbg_rstd"][:], rstd[:]
                        )
                    # the folded mean term rides pre-rstd: the drain scale
                    # multiplies (FFraw - mean*colsum(wff)) by rstd
                    rstdB = s3.tile([128, S], BF16, name="rstdB")
                    nc.gpsimd.partition_broadcast(rstdB[:], rstd[:])

                    # FF contracts RAW at (LN folded: rstd scales the PSUM at
                    # drain; the mean term rides as a K=1 accumulation row)
                    for m in range(NT):
                        ff = psU.tile([128, S], F32, name="ff", tag="u")
                        for qh in range(2):
                            sl = slice(512 * qh, 512 * (qh + 1))
                            for kc in range(NT):
                                nc.tensor.matmul(
                                    ff[:, sl],
                                    wffr[kc][:, 128 * m : 128 * (m + 1)],
                                    aT[kc][:, sl],
                                    start=kc == 0, stop=False,
                                )
                            nc.tensor.matmul(
                                ff[:, sl],
                                negwcol[:, 128 * m : 128 * (m + 1)],
                                mean[:, sl],
                                start=False, stop=True,
                            )
                        t1 = w3.tile([128, S], BF16, name="t1", tag="t1")
                        nc.vector.tensor_mul(t1[:], ff[:], rstdB[:])
                        y = w3.tile([128, S], F32, name="y", tag="y")
                        nc.vector.scalar_tensor_tensor(
                            y[:], t1[:], gb[m], aT[m][:],
                            op0=OP.add, op1=OP.add,
                        )
                        nc.sync.dma_start(out_d[128 * m : 128 * (m + 1), :], y[:])

        if loop_n is not None:
            # unroll bodies per loop trip so the tile framework pipelines
            # across iterations (loop-boundary stalls amortized); total
            # iterations stay loop_n
            assert loop_n % unroll == 0
            with tc.For_i(0, loop_n // unroll, 1) as i:
                for _ in range(unroll):
                    body(i)
        else:
            for _ in range(unroll):
                body()
        for cm in reversed(pools_cm):
            cm.__exit__(None, None, None)
        cpool_cm.__exit__(None, None, None)

    # force the single act table set containing both Exp and Ln so the
    # loop never pays a table switch
    import concourse.bacc as bacc_mod

    orig_get = bacc_mod.get_activation_tables

    def only_ln_exp(arch):
        # keep list order/ids intact; strip our functions from other sets so
        # the chooser is forced to the combined set (id preserved)
        tabs = orig_get(arch)
        ours = {AF.Exp, AF.Ln, AF.Square}
        return {
            k: (v if k == "natural_log_exp_and_others" else v - ours)
            for k, v in tabs.items()
        }

    bacc_mod.get_activation_tables = only_ln_exp
    try:
        nc.compile()
    finally:
        bacc_mod.get_activation_tables = orig_get
    return nc


def prep_inputs(x, wq, bq, wk, bk, wv, bv, wp, bp, gamma, beta, wff, bff):
    x = np.asarray(x, dtype=np.float32)
    wq = np.asarray(wq, np.float32)
    bq = np.asarray(bq, np.float32)
    wk = np.asarray(wk, np.float32)
    bk = np.asarray(bk, np.float32)
    wv = np.asarray(wv, np.float32)
    bv = np.asarray(bv, np.float32)
    wp_ = np.asarray(wp, np.float32)
    bp = np.asarray(bp, np.float32)
    gamma = np.asarray(gamma, np.float32)
    beta = np.asarray(beta, np.float32)
    wff = np.asarray(wff, np.float32)
    bff = np.asarray(bff, np.float32)

    bf = ml_dtypes.bfloat16
    scale = np.float32(1.0 / np.sqrt(np.float32(DH)))
    wq2 = np.concatenate([wq * scale, wq * scale], axis=0).astype(bf)  # [128,64]
    wk2 = np.concatenate([wk, wk], axis=0).astype(bf)  # [128,64]
    wv65 = np.concatenate([wv, np.zeros((DH, 1), np.float32)], axis=1)  # [64,65]
    wv2 = np.concatenate([wv65, wv65], axis=0).astype(bf)  # [128,65]
    wp2 = wp_.astype(bf)  # [64,64]
    bq2 = (np.concatenate([bq, bq]).reshape(128, 1) * scale).astype(np.float32)
    bpp = bv @ wp_ + bp
    bp2 = np.concatenate([bpp, bpp]).reshape(128, 1).astype(np.float32)

    # channel permutation: head-major c' = h*64+dh holds original c = dh*12+h
    cp = np.arange(D)
    hh, dd = cp // 64, cp % 64
    p = dd * H + hh
    wffg = wff * gamma[:, None]
    bffg = bff + beta @ wff
    wffp = np.ascontiguousarray(wffg[p][:, p]).astype(bf)
    bffp = np.ascontiguousarray(bffg[p].reshape(NT, 128).T).astype(np.float32)
    onesb = np.ones((128, 1), bf)
    # column sums of the (permuted, gamma-folded) FF weights, negated:
    # FF(normed) = rstd*(wff.T@at) - (mean*rstd)*colsum(wff)
    negwcol = (-wffp.astype(np.float32).sum(axis=0)).reshape(1, D).astype(bf)

    shared = {
        "wq2": wq2, "wk2": wk2, "wv2": wv2, "wp2": wp2,
        "bq2": bq2, "bp2": bp2,
        "wffp": wffp, "bffp": bffp, "onesb": onesb, "negwcol": negwcol,
    }
    in_maps = []
    for i in range(x.shape[0]):
        m = dict(shared)
        m["xT"] = np.ascontiguousarray(x[i].T).astype(bf)
        in_maps.append(m)
    return in_maps, p


def postprocess(results, p):
    outs = []
    for r in results:
        yt = r["out"].T
        y = np.empty_like(yt)
        y[:, p] = yt
        outs.append(y)
    return np.stack(outs)


def kernel(**inputs) -> np.ndarray:
    if "nc" not in _CACHE:
        _CACHE["nc"] = build_nc()
    nc = _CACHE["nc"]
    in_maps, p = prep_inputs(**inputs)
    res = run_bass_kernel_spmd(nc, in_maps, list(range(8)))
    return postprocess(res.results, p)
